# revision 6
# baseline (speedup 1.0000x reference)
"""Trainium2 Bass kernel for nn_MultiHeadAttention_7584912245188.

Reference computes (no softmax!):
    qkv = x @ Wqkv + bqkv ; split q,k,v ; per head: y = (q k^T / sqrt(D)) v
    out = y @ Wff + bff

No softmax => attention is linear and reassociates: (Q K^T) V = Q (K^T V).
With X_aug = [X | 1] ([N, 97]) and G = X_aug^T X_aug ([97, 97]):
    out = X_aug @ Wfin,   Wfin = sum_h P_h G Q_h + e_last bff^T
    P_h = Wq_aug_h Wk_aug_h^T [97,97],  Q_h = D^-0.5 Wv_aug_h Wff_h [97,96]
P_h / Q_h host-precomputed.  O(N*E^2) instead of O(N^2*D).  Per batch on
device: G (16 accumulating matmuls over 128-row chunks), R = G @ Qcat in
two 3-head slices, Wfin accumulation (bias + 6 head matmuls, one PSUM
group), then out chunks = X_chunk @ Wfin with lhsT taken from a
host-shipped transposed copy of this core's x half (no PE transposes).

Latency-driven schedule (cost-model timeline, engine-validated on HW):
  - inputs: XA (first x half) on SP HWDGE grabs the first DMA transfer
    slot; XB via a plain SWDGE dma_start queues right behind it; wpack is
    split in two tiles/DMAs so R's reads only wait the Qcat columns (tile
    dependency tracking is tile-granular); xt last (finals-only).
  - all PSUM->SBUF casts on DVE (lowest completion latency) except the
    second R slice and the big middle output group, which run on the
    otherwise-idle Activation engine in parallel. GPSIMD/Pool cannot read
    PSUM (BIR verifier), so it only handles SWDGE preps and memsets.
  - output via kv_writeback (SWDGE): the prep is emitted BEFORE any
    writer of the staging buffer exists, so it carries no data waits and
    its ~1us descriptor-gen pre-runs on the idle Pool engine; its
    cost-model descriptors cover 16 rows each, so the tail is just
    trigger + ~70ns transfer + completion sem. Ordering: WAR edges
    (staging copy -> prep deferred read) are removed and replaced by
    explicit sync edges from every staging copy onto the trigger.
  - output staging is split {1, 4, 3} chunks over DVE/Act/DVE so the
    first group's cast starts the moment its pair of finals retires and
    the tail-critical last group rides DVE.

Sharding (8 cores): core c -> (batch b = c//2, sequence half h = c%2).
Each core receives x[b] (ones column appended host-side) rolled so "its"
half comes first, computes G from the full batch (redundantly within the
pair - cheaper than a collective), and writes only its half of the output
rows. Row layout within a half: row = 8*p + j (p = partition, j = 0..7),
one contiguous ~1.5KB run per partition per DMA.
"""

import numpy as np
from contextlib import ExitStack

import concourse.bass as bass
import concourse.tile as tile
from concourse import bacc, mybir
from concourse import bass_utils

B, N, E = 4, 2048, 96
H = 6
D = E // H            # 16
P = 128
NCH = N // P          # 16 chunks of 128 rows
HALF = NCH // 2       # 8 chunks per core
EA = E + 1            # 97 (augmented with ones column)
SCALE = float(D) ** -0.5
F32 = mybir.dt.float32
F16 = mybir.dt.float16

# wpack (fp16, 97 partitions) column layout: Qcat | PcatT | onehot | bff
C_Q = 0
C_P = C_Q + H * E            # 576
C_OH = C_P + H * EA          # 1158
C_BF = C_OH + EA             # 1255
WPACK_COLS = C_BF + E        # 1351
R_OH = C_OH - C_P            # offsets within the wr tile
R_BF = C_BF - C_P

N_CORES = 8

_NC_CACHE = {}
LAST_RESULTS = None


def _build_nc():
    nc = bacc.Bacc(
        "TRN2", target_bir_lowering=False, debug=False, num_devices=N_CORES
    )
    # The framework preamble emits four const-AP memsets serialized on the
    # Pool engine; their drain gates the initial all-engine barrier (and so
    # the first input DMA). Spread them across the otherwise-idle DVE/Act/
    # SP engine streams so they run concurrently.
    _pre_ms = [i for i in nc.m.functions[0].blocks[0].instructions
               if type(i).__name__ == "InstMemset"]
    for _i in _pre_ms[2:]:
        _i.engine = mybir.EngineType.DVE
    x = nc.dram_tensor("x", [N, EA], F16, kind="ExternalInput").ap()
    xt = nc.dram_tensor("xt", [EA, HALF * P], F16, kind="ExternalInput").ap()
    wpacki = nc.dram_tensor("wpack", [EA, WPACK_COLS], F16, kind="ExternalInput").ap()
    out = nc.dram_tensor("out", [N // 2, E], F16, kind="ExternalOutput").ap()

    with tile.TileContext(nc) as tc, ExitStack() as ctx:
        consts = ctx.enter_context(tc.tile_pool(name="consts", bufs=1))
        big = ctx.enter_context(tc.tile_pool(name="big", bufs=1))
        small = ctx.enter_context(tc.tile_pool(name="small", bufs=1))
        ps_g = ctx.enter_context(tc.tile_pool(name="ps_g", bufs=1, space="PSUM"))
        ps_r = ctx.enter_context(tc.tile_pool(name="ps_r", bufs=1, space="PSUM"))
        ps_w = ctx.enter_context(tc.tile_pool(name="ps_w", bufs=1, space="PSUM"))
        ps_o = ctx.enter_context(tc.tile_pool(name="ps_o", bufs=1, space="PSUM"))

        # --- loads. The x half this core computes G from last (XB) is
        # split in two so its first four chunks land one transfer-slot
        # earlier: XA on SP HWDGE takes the first DMA slot, XB1 via SWDGE
        # queues right behind, XB2 via Act HWDGE third. wr rides the SWDGE
        # queue (prep pre-runs), wq/XT take the second HWDGE slots.
        xh = x.rearrange("(h p j) e -> h p j e", h=2, j=HALF)
        XA = big.tile([P, HALF, EA], F16)
        nc.sync.dma_start(out=XA[:], in_=xh[0])                   # SP HWDGE
        XB1 = big.tile([P, HALF // 2, EA], F16)
        nc.gpsimd.dma_start(out=XB1[:], in_=xh[1][:, 0 : HALF // 2])   # SWDGE
        XB2 = big.tile([P, HALF // 2, EA], F16)
        nc.scalar.dma_start(out=XB2[:], in_=xh[1][:, HALF // 2 :])     # Act HWDGE
        wr = consts.tile([EA, WPACK_COLS - C_P], F16)
        nc.gpsimd.dma_start(out=wr[:], in_=wpacki[:, C_P:])       # SWDGE
        wq = consts.tile([EA, C_P], F16)
        nc.sync.dma_start(out=wq[:], in_=wpacki[:, C_Q:C_P])      # SP HWDGE 2
        XT = big.tile([EA, HALF, P], F16)
        nc.scalar.dma_start(out=XT[:], in_=xt.rearrange("e (j p) -> e j p", j=HALF))
        # ctx index row for the kv_writeback output path (all zeros)
        kvidx = consts.tile([P, 1], mybir.dt.int32)
        nc.gpsimd.memset(kvidx[:], 0)

        # --- output staging buffer + kv_writeback prep, emitted BEFORE any
        # writer of osb so the prep carries no data waits and its desc-gen
        # pre-runs on the idle Pool engine. The trigger comes last.
        osb = big.tile([P, HALF, E], F16)
        kv_sem = nc.alloc_semaphore("kv_out_dma")
        kv_prep = nc.gpsimd.kv_writeback(
            out.rearrange("(one p j) e -> one p j e", j=HALF, one=1),
            osb[:].rearrange("p (j one) e -> p j one e", one=1),
            kvidx[:],
            prepare_only=True,
            sem=kv_sem,
        )
        # Tile attaches its own DMA-completion sem to the prep; cost model
        # and HW fire on_update[0] as the entry's completion sem, so drop
        # ours to leave Tile's as the single source of truth.
        kv_prep.ins.sync_info.on_update = []

        def Xc(c):
            if c < HALF:
                return XA[:, c, :]
            if c < HALF + HALF // 2:
                return XB1[:, c - HALF, :]
            return XB2[:, c - HALF - HALF // 2, :]

        # --- G = X_aug^T X_aug: one 16-matmul PSUM accumulation group;
        # the XA chunks pipeline behind XA's arrival while XB is in
        # flight, then a single DVE cast stages it to fp16.
        g_ps = ps_g.tile([EA, EA], F32, tag="g", name="g")
        for c in range(NCH):
            nc.tensor.matmul(
                g_ps[:], lhsT=Xc(c), rhs=Xc(c),
                start=(c == 0), stop=(c == NCH - 1),
            )
        g_h = small.tile([EA, EA], F16)
        nc.vector.tensor_copy(out=g_h[:], in_=g_ps[:])

        # --- R = G @ Qcat in two 3-head slices; casts on DVE / Act in
        # parallel (distinct PSUM tiles so they don't serialize).
        RW = 3 * E                        # 3 heads per R slice
        r_h = [small.tile([EA, RW], F16, name=f"r{i}_h", tag=f"r{i}_h") for i in range(2)]
        r_cp = [nc.vector.tensor_copy, nc.scalar.copy]
        for i in range(2):
            r_ps = ps_r.tile([EA, RW], F32, tag=f"r{i}", name=f"r{i}")
            nc.tensor.matmul(
                r_ps[:], lhsT=g_h[:], rhs=wq[:, i * RW : (i + 1) * RW],
                start=True, stop=True,
            )
            r_cp[i](out=r_h[i][:], in_=r_ps[:])

        def rh(h):
            return r_h[h // 3], (h % 3) * E

        # --- Wfin = e_last bff^T + sum_h P_h R_h (one PSUM accumulation
        # group; bias matmul first: it only needs wpack, which lands well
        # before the R slices).
        wf_ps = ps_w.tile([EA, E], F32, tag="wf", name="wf")
        for h in range(H):
            lt = wr[:, h * EA : (h + 1) * EA]
            rt, ro = rh(h)
            nc.tensor.matmul(
                wf_ps[:], lhsT=lt, rhs=rt[:, ro : ro + E],
                start=(h == 0), stop=(h == H - 1),
            )
            if h == H // 2 - 1:
                # bias in the PE idle gap between the two head groups (the
                # second group waits its R slice). Not first: its ldweights
                # would park the PE queue on wr (a late DMA slot) ahead of
                # the R matmuls; not last: it would push the group's stop
                # 40ns later. WAW edges within the PSUM group keep order.
                nc.tensor.matmul(
                    wf_ps[:], lhsT=wr[0:1, R_OH : R_OH + EA],
                    rhs=wr[0:1, R_BF : R_BF + E], start=False, stop=False,
                )
        wf_h = small.tile([EA, E], F16)
        nc.vector.tensor_copy(out=wf_h[:], in_=wf_ps[:])

        # --- finals: out chunk = X_chunk @ Wfin via lhsT = XT chunk, in
        # three PSUM groups staged to fp16 as their matmuls complete. The
        # first chunk's cast (DVE) starts immediately; Act absorbs the big
        # middle group; the tail-critical last group rides DVE again.
        og_sizes = [1, 4, 3]
        og_home = [(ps_g, "g"), (ps_o, "og1"), (ps_o, "og2")]
        
        cp = [nc.vector.tensor_copy, nc.scalar.copy, nc.vector.tensor_copy]
        og_copies = []
        base = 0
        for grp in range(3):
            w = og_sizes[grp]
            opool, otag = og_home[grp]
            og = opool.tile([P, w, E], F32, tag=otag, name=f"og{grp}")
            for j in range(w):
                nc.tensor.matmul(
                    og[:, j, :], lhsT=XT[:, base + j, :], rhs=wf_h[:],
                    start=True, stop=True,
                )
            c = cp[grp](out=osb[:, base : base + w, :], in_=og[:])
            # Tile adds a WAR edge copy -> prep (writer after the prep's
            # deferred read of osb), which materializes as a wait on the
            # DMA completion -- circular with the trigger gating below.
            # The explicit trigger ordering makes it unnecessary: drop it.
            c.ins.try_remove_dependency(kv_prep.ins.name)
            og_copies.append(c)
            base += w

        # --- fire the output DMA once every staging copy completed, via
        # explicit sync edges straight onto the trigger.
        import bass_rust as _br
        trig = nc.gpsimd.trigger_dma(count=None)
        for c in og_copies:
            trig.ins.add_dependency(c.ins.name, _br.DependencyInfo.SYNC_ONLY)

    nc.compile()
    return nc


def get_nc():
    if "nc" not in _NC_CACHE:
        _NC_CACHE["nc"] = _build_nc()
    return _NC_CACHE["nc"]


def _host_weights(Wqkv, bqkv, Wff, bff):
    waug = np.concatenate(
        [np.asarray(Wqkv, np.float64), np.asarray(bqkv, np.float64)[None, :]], axis=0
    )
    Wq, Wk, Wv = waug[:, 0:E], waug[:, E : 2 * E], waug[:, 2 * E : 3 * E]
    Wff = np.asarray(Wff, np.float64)
    wp = np.zeros((EA, WPACK_COLS), np.float16)
    for h in range(H):
        hd = slice(h * D, (h + 1) * D)
        Ph = Wq[:, hd] @ Wk[:, hd].T                    # [97, 97]
        Qh = SCALE * (Wv[:, hd] @ Wff[hd, :])           # [97, 96]
        wp[0:EA, C_P + h * EA : C_P + (h + 1) * EA] = Ph.T.astype(np.float16)
        wp[0:EA, C_Q + h * E : C_Q + (h + 1) * E] = Qh.astype(np.float16)
    wp[0, C_OH + E] = 1.0                               # e_last selector row
    wp[0, C_BF : C_BF + E] = np.asarray(bff, np.float16)
    return {"wpack": wp}


def make_in_maps(x, Wqkv, bqkv, Wff, bff):
    x = np.asarray(x, np.float32)
    w = _host_weights(Wqkv, bqkv, Wff, bff)
    ones = np.ones((N, 1), np.float16)
    x16 = x.astype(np.float16)
    in_maps = []
    for c in range(N_CORES):
        b, h = divmod(c, 2)
        xb = x16[b]
        if h:
            xb = np.concatenate([xb[N // 2 :], xb[: N // 2]], axis=0)
        xa = np.ascontiguousarray(np.concatenate([xb, ones], axis=1))
        # xt[e, j*P + p] = my_half[8p + j, e]  (matches XT[e, j, p])
        xtm = np.ascontiguousarray(
            xa[: N // 2].reshape(P, HALF, EA).transpose(2, 1, 0).reshape(EA, -1)
        )
        m = {"x": xa, "xt": xtm}
        m.update(w)
        in_maps.append(m)
    return in_maps


def assemble(results):
    out = np.empty((B, N, E), np.float32)
    for c in range(N_CORES):
        b, h = divmod(c, 2)
        out[b, h * (N // 2) : (h + 1) * (N // 2)] = results[c]["out"]
    return out


def kernel(x, Wqkv, bqkv, Wff, bff):
    global LAST_RESULTS
    nc = get_nc()
    in_maps = make_in_maps(x, Wqkv, bqkv, Wff, bff)
    res = bass_utils.run_bass_kernel_spmd(
        nc, in_maps, core_ids=list(range(N_CORES))
    )
    LAST_RESULTS = res
    return assemble(res.results)
